# revision 42
# baseline (speedup 1.0000x reference)
"""Trainium2 Bass kernel for nn_MultiHeadAttention_73607149519012.

MHA: B=8, S=1024, D=1024, H=16 heads, depth=64, fp32 in/out.
Sharding: data-parallel over batch -- one batch element per NeuronCore (8).

The attention path runs in fp8e4 (TRN E4M3, max 240); only q_in @ Wo_top
stays bf16 since it dominates the output magnitude (ctx contributes ~2%).
Matmul mode choices are driven by HW measurements (per-instr cost is
N_moving rows x 1 cycle at 2.4GHz regardless of K/dtype; fp8 DoubleRow
contracts 2 K-groups per instruction at the same cost):
  - QKV projections, PV, ctx-out-proj: fp8 DoubleRow with K=128 groups
    (full rate, halves instruction count vs bf16)
  - logits: plain fp8 K=64 matmuls -- K=32 DoubleRow would halve the
    instruction count but K=32 matmuls run at half clock (HAM gate)
  - q_in @ Wo_top: bf16 (precision-critical half of the output)
Exp runs on ACT in (128,1024) chunks reading logits psum directly, with
the additive mask folded in as the per-partition activation bias.

ctx layout: head h -> quad q=h//4, half lh=(h//2)%2, parity par=h%2.
The PV stationary V-tile places head payload at array cols 0-63 (lh=0,
rowsum col 64) or cols 64-127 (lh=1, rowsum col 0), so ctx lands at psum
partitions lh*64..lh*64+63 and ctxP[q] holds 4 heads on 128 partitions
(par as the DR slot).  The output projection ctx @ Wo_bot then contracts
K=256 per DR matmul across all 128 array rows: half the instructions of
a 64-partition layout AND no K<=64 stream to trip the HAM half-clock
gate (the old psoB tail ran at ~2x slower issue in ham k=4 state).

Pipeline: projections and attention are software-pipelined per head with
~1-2us filler units (PV of an older head, outA chunks, partial
ctx-out-proj) interleaved between logits chunks, so the in-order PE queue
always has independent work while a logits matmul waits for ACT to free
an lps buffer.  ctx@Wo_bot splits as psoH1 (quads 0,1 = heads 0-7,
accumulated into oa during heads 12-15) and psoH2 (quads 2,3, tail).

Scaling: weights pre-scaled x16 on host; Q,K,V carry 16x; logits psum is
256x so the exp scale is (1/8)/256; ctx is 16x; Wo_bot is 16x so the ctx
half of the output projection is 256x, descaled in the final fused
(psum*1/256)+outA op.

DMA: merged need-order descriptors on the sync ring (the per-ring
descriptor issue is ~0.65us, so fewer descriptors in need-order get the
critical path loaded earlier); small/late loads ride the scalar ring.

Layouts (host-prepped):
  mq/mk/mv [128,2,4,2,1024] : slot0 = W^T din-pair tiles (x16 fp8),
                              slot1 = x^T din-pair tiles (fp8)
  (wv dout columns permuted so each 512-half holds lh=0 / lh=1 heads)
  qt/kt (8 ea) [128,1024]   : Q^T,K^T dout-tiles (partition=dout%128)
  v65[t]   [128,2,4,2,2,128]: V kpos-pair tiles [kp,ks,q,lh,par,col];
                              payload at cols lh*64.., ones col at
                              64*(1-lh) accumulating the softmax row-sum
  pt (5 bufs) [128,8,1024]  : p^T = exp(logits) fp8, per head
  ctxP[q]     [128,2,1024]  : 16*ctx^T quad tiles (4 heads each)
  wo2         [128,4,2,1024]: Wo bottom rows matching ctxP
  mba [128,2,8,1024] bf16   : q_in^T / Wo top rows for the bf16 half
"""

import os
import numpy as np

import concourse.mybir as mybir
import concourse.tile as tile
from concourse import bacc
from concourse.bass_utils import run_bass_kernel_spmd

F32 = mybir.dt.float32
BF16 = mybir.dt.bfloat16
FP8 = mybir.dt.float8e4
I8 = mybir.dt.int8
AF = mybir.ActivationFunctionType
ALU = mybir.AluOpType
DR = mybir.MatmulPerfMode.DoubleRow
NP8 = mybir.dt.np(FP8)
NPB = mybir.dt.np(BF16)

D = 1024
S = 1024
H = 16
B = 8
SW = 16.0
C_EXP = 0.125 / (SW * SW)
C_PH3 = 1.0 / (SW * SW)

LAST_EXEC_NS = None
LAST_RES = None


def build_nc():
    nc = bacc.Bacc(None, target_bir_lowering=False)

    mq_d = nc.dram_tensor("mq", [128, 2, 4, 2, 1024], FP8, kind="ExternalInput")
    mk_d = nc.dram_tensor("mk", [128, 2, 4, 2, 1024], FP8, kind="ExternalInput")
    mv_d = nc.dram_tensor("mv", [128, 2, 4, 2, 1024], FP8, kind="ExternalInput")
    mb3_d = nc.dram_tensor("mb3", [128, 24], F32, kind="ExternalInput")
    bvb_d = nc.dram_tensor("bvb", [128, 1024], F32, kind="ExternalInput")
    mba_d = nc.dram_tensor("mba", [128, 2, 8, 1024], BF16, kind="ExternalInput")
    wo2_d = nc.dram_tensor("wo2", [128, 4, 2, 1024], FP8, kind="ExternalInput")
    bob_d = nc.dram_tensor("bob", [128, 1024], F32, kind="ExternalInput")
    out_d = nc.dram_tensor("out", [S, D], F32, kind="ExternalOutput")

    with tile.TileContext(nc) as tc:
        with (
            tc.tile_pool(name="cst", bufs=1) as cst,
            tc.tile_pool(name="qkp", bufs=1) as qkp,
            tc.tile_pool(name="vp", bufs=1) as vp,
            tc.tile_pool(name="ptp", bufs=1) as ptp,
            tc.tile_pool(name="ctxp", bufs=1) as ctxp,
            tc.tile_pool(name="oap", bufs=1) as oap,
            tc.tile_pool(name="rp", bufs=2) as rp,
            tc.tile_pool(name="pvcp", bufs=2) as pvcp,
            tc.tile_pool(name="outp", bufs=3) as outp,
            tc.tile_pool(name="p2a", bufs=1) as p2a,
            tc.tile_pool(name="lpsp", bufs=2, space="PSUM") as lpsp,
            tc.tile_pool(name="pvp", bufs=2, space="PSUM") as pvp,
            tc.tile_pool(name="oaps", bufs=2, space="PSUM") as oaps,
        ):
            mb3_sb = cst.tile([128, 24], F32, name="mb3")
            bq2_sb = mb3_sb[:, 0:8]
            bk2_sb = mb3_sb[:, 8:16]
            mb_sb = mb3_sb[:, 16:24]
            bvb_sb = cst.tile([128, 1024], F32, name="bvb")
            bob_sb = cst.tile([128, 1024], F32, name="bob")

            qt = [qkp.tile([128, 1024], FP8, name=f"qt_{i}") for i in range(8)]
            kt = [qkp.tile([128, 1024], FP8, name=f"kt_{i}") for i in range(8)]
            # [kpos-part, kpos-slot, quad, lh, par, col]
            v65 = [vp.tile([128, 2, 4, 2, 2, 128], FP8, name=f"v65_{i}")
                   for i in range(4)]
            pts = [ptp.tile([128, 8, 1024], FP8, name=f"pt{i}") for i in range(5)]
            ctxP = [ctxp.tile([128, 2, 1024], FP8, name=f"ctxP_{i}")
                    for i in range(4)]
            oa = [oap.tile([128, 1024], BF16, name=f"oa{i}") for i in range(8)]

            pcount = [0]

            def proj_psum(nm):
                pool, tg = ((pvp, "ps0"), (oaps, "ps1"))[pcount[0] % 2]
                t = pool.tile([128, 512], F32, name=nm, tag=tg)
                pcount[0] += 1
                return t

            def emit_qk_proj(a, w_sb, x_sb, dst, bias):
                # dout-tile a: out (128 dout, 1024 s) as 2 chunks of 512
                for sc in range(2):
                    ps = proj_psum(f"ps_{dst[a].name}_{sc}")
                    for t in range(4):
                        nc.tensor.matmul(
                            ps, w_sb[:, t, :, a * 128:(a + 1) * 128],
                            x_sb[:, t, :, sc * 512:(sc + 1) * 512],
                            start=(t == 0), stop=(t == 3), perf_mode=DR)
                    nc.vector.tensor_scalar_add(
                        dst[a][:, sc * 512:(sc + 1) * 512], ps,
                        bias[:, a:a + 1])

            def emit_v_proj(st2, wv_sb, xv_sb):
                # wv columns permuted: dc half = lh group, within it
                # (quad, par, depth) order
                t2, sl = st2 // 2, st2 % 2
                for dc in range(2):
                    ps = proj_psum(f"ps_v_{st2}_{dc}")
                    for t in range(4):
                        nc.tensor.matmul(
                            ps, xv_sb[:, t, :, st2 * 128:(st2 + 1) * 128],
                            wv_sb[:, t, :, dc * 512:(dc + 1) * 512],
                            start=(t == 0), stop=(t == 3), perf_mode=DR)
                    nc.vector.tensor_add(
                        v65[t2][:, sl, :, dc, :, dc * 64:dc * 64 + 64],
                        ps.rearrange("p (q r e) -> p q r e", r=2, e=64),
                        bvb_sb[:, dc * 512:(dc + 1) * 512].rearrange(
                            "p (q r e) -> p q r e", r=2, e=64))

            def emit_head(h, fillers=()):
                """logits (K=64 plain matmuls) + exp -> pt tile.
                fillers: callables emitting ~1-2us of independent PE work,
                interleaved between logits chunks so the in-order PE queue
                has work while a logits matmul waits for ACT to free an
                lps buffer."""
                a, base = h // 2, (h % 2) * 64
                ptt = pts[h % 5]
                fill = list(fillers)
                for kpt in range(8):
                    lps = lpsp.tile([128, 1024], F32, name=f"lps_{h}_{kpt}",
                                    tag="lps")
                    for half in range(2):
                        nc.tensor.matmul(
                            lps[:, half * 512:(half + 1) * 512],
                            kt[a][base:base + 64, kpt * 128:(kpt + 1) * 128],
                            qt[a][base:base + 64, half * 512:(half + 1) * 512],
                            start=True, stop=True, tile_position=(base, 0))
                    nc.scalar.activation(ptt[:, kpt, :], lps, AF.Exp,
                                         bias=mb_sb[:, kpt:kpt + 1],
                                         scale=C_EXP)
                    if kpt % 2 == 1 and fill:
                        fill.pop(0)()
                while fill:
                    fill.pop(0)()
                return ptt

            def emit_pv_qc(h, qc):
                ptt = pts[h % 5]
                q4, lh, par = h // 4, (h // 2) % 2, h % 2
                ncol = 65 if lh == 0 else 128
                pv = pvp.tile([128, 512], F32, name=f"pv_{h}_{qc}", tag="ps0")
                for t in range(4):
                    nc.tensor.matmul(
                        pv[0:ncol, :], v65[t][:, :, q4, lh, par, 0:ncol],
                        ptt[:, 2 * t:2 * t + 2, qc * 512:(qc + 1) * 512],
                        start=(t == 0), stop=(t == 3), perf_mode=DR)
                # two copies (payload to a side buffer + rowsum to
                # partition 0 for the custom-DVE reciprocal, which can
                # read neither PSUM nor partition-offset slices) free the
                # psum in ~1.4us instead of holding it through the ~3us
                # recip/broadcast/mul chain, so back-to-back PV units
                # don't convoy on the two ps0 psum buffers
                rs_lo = 64 * (1 - lh)
                cx = slice(lh * 64, lh * 64 + 64)
                pvc = pvcp.tile([128, 512], F32, name=f"pvc_{h}_{qc}",
                                tag="pvc")
                nc.vector.tensor_copy(pvc[cx], pv[cx, :])
                rs = rp.tile([1, 512], F32, name=f"rs_{h}_{qc}", tag="rs")
                nc.vector.tensor_copy(rs, pv[rs_lo:rs_lo + 1, :])
                rc = rp.tile([1, 512], F32, name=f"rc_{h}_{qc}", tag="rc")
                nc.vector.reciprocal_approx_fast(rc, rs)
                rbc = rp.tile([128, 512], F32, name=f"rbc_{h}_{qc}", tag="rbc")
                nc.gpsimd.partition_broadcast(rbc, rc, channels=128)
                nc.vector.tensor_mul(
                    ctxP[q4][cx, par, qc * 512:(qc + 1) * 512],
                    pvc[cx], rbc[cx])

            def emit_pv(h):
                emit_pv_qc(h, 0)
                emit_pv_qc(h, 1)

            def emit_oa(idx, xqb_sb, woA_sb):
                st, dc = idx // 2, idx % 2
                ps = oaps.tile([128, 512], F32, name=f"psA_{st}_{dc}", tag="ps1")
                for kt2 in range(8):
                    nc.tensor.matmul(
                        ps, xqb_sb[:, kt2, st * 128:(st + 1) * 128],
                        woA_sb[:, kt2, dc * 512:(dc + 1) * 512],
                        start=(kt2 == 0), stop=(kt2 == 7))
                nc.vector.tensor_add(oa[st][:, dc * 512:(dc + 1) * 512], ps,
                                     bob_sb[:, dc * 512:(dc + 1) * 512])

            xqwoA_sb = p2a.tile([128, 2, 8, 1024], BF16, name="xqwoA_sb")
            xqb_sb = xqwoA_sb[:, 0]
            woA_sb = xqwoA_sb[:, 1]

            # ============ phase 1 + attention, software-pipelined ============
            with tc.tile_pool(name="p1", bufs=1) as p1:
                mq_sb = p1.tile([128, 2, 4, 2, 1024], FP8, name="mq_sb")
                mk_sb = p1.tile([128, 2, 4, 2, 1024], FP8, name="mk_sb")
                mv_sb = p1.tile([128, 2, 4, 2, 1024], FP8, name="mv_sb")
                wq_sb, xq_sb = mq_sb[:, 0], mq_sb[:, 1]
                wk_sb, xk_sb = mk_sb[:, 0], mk_sb[:, 1]
                wv_sb, xv_sb = mv_sb[:, 0], mv_sb[:, 1]

                # sync-ring descriptors in need order; small/late loads on
                # the scalar ring
                nc.sync.dma_start(mq_sb, mq_d[...])
                nc.sync.dma_start(mk_sb, mk_d[...])
                nc.sync.dma_start(mv_sb, mv_d[...])
                nc.sync.dma_start(xqwoA_sb, mba_d[...])
                nc.scalar.dma_start(mb3_sb, mb3_d[:, :])
                nc.scalar.dma_start(bvb_sb, bvb_d[:, :])

                # warm the ACT exp table before any data arrives
                warm = rp.tile([1, 8], F32, name="warm", tag="warm")
                nc.scalar.activation(warm, bq2_sb[0:1, :], AF.Exp, scale=0.0)

                for t in range(4):
                    # softmax row-sum ones column: col 64 for lh=0 heads,
                    # col 0 for lh=1 heads; cols 1-63 of lh=1 heads are
                    # read by the 128-col stationary but unused (their psum
                    # partitions are ignored) -- zero once so the fp8 bits
                    # are finite
                    nc.vector.memset(v65[t][:, :, :, 0, :, 64:65], 1.0)
                    nc.vector.memset(v65[t][:, :, :, 1, :, 0:1], 1.0)
                    nc.gpsimd.memset(v65[t][:, :, :, 1, :, 1:64], 0.0)

                # h0/h1 need qt[0], kt[0]
                emit_qk_proj(0, wq_sb, xq_sb, qt, bq2_sb)
                emit_qk_proj(0, wk_sb, xk_sb, kt, bk2_sb)
                emit_head(0)
                emit_qk_proj(1, wq_sb, xq_sb, qt, bq2_sb)
                emit_qk_proj(1, wk_sb, xk_sb, kt, bk2_sb)
                emit_head(1)
                for st2 in range(4):
                    emit_v_proj(st2, wv_sb, xv_sb)
                emit_head(2)
                emit_qk_proj(2, wq_sb, xq_sb, qt, bq2_sb)
                emit_qk_proj(2, wk_sb, xk_sb, kt, bk2_sb)
                for st2 in range(4, 8):
                    emit_v_proj(st2, wv_sb, xv_sb)
                emit_head(3)
                for a in range(3, 8):
                    emit_qk_proj(a, wq_sb, xq_sb, qt, bq2_sb)
                    emit_qk_proj(a, wk_sb, xk_sb, kt, bk2_sb)

            # =================== main attention loop ===================
            with tc.tile_pool(name="p2", bufs=1) as p2:
                wo2_sb = p2.tile([128, 4, 2, 1024], FP8, name="wo2_sb")
                nc.scalar.dma_start(wo2_sb, wo2_d[:, :, :, :])
                nc.scalar.dma_start(bob_sb, bob_d[:, :])

                def emit_pso(idx, qlo, into_oa):
                    # ctx @ Wo_bot, quads (qlo, qlo+1): K=256 per DR matmul
                    # across all 128 array rows
                    st, dc = idx // 2, idx % 2
                    ps = oaps.tile([128, 512], F32, name=f"pso{qlo}_{st}_{dc}",
                                   tag="ps1")
                    for q in (qlo, qlo + 1):
                        nc.tensor.matmul(
                            ps, ctxP[q][:, :, st * 128:(st + 1) * 128],
                            wo2_sb[:, q, :, dc * 512:(dc + 1) * 512],
                            start=(q == qlo), stop=(q == qlo + 1),
                            perf_mode=DR)
                    if into_oa:
                        nc.vector.scalar_tensor_tensor(
                            oa[st][:, dc * 512:(dc + 1) * 512], ps, C_PH3,
                            oa[st][:, dc * 512:(dc + 1) * 512],
                            ALU.mult, ALU.add)
                    else:
                        ot = outp.tile([128, 512], F32, name=f"ot_{st}_{dc}",
                                       tag="ot")
                        nc.vector.scalar_tensor_tensor(
                            ot, ps, C_PH3, oa[st][:, dc * 512:(dc + 1) * 512],
                            ALU.mult, ALU.add)
                        nc.sync.dma_start(
                            out_d[st * 128:(st + 1) * 128,
                                  dc * 512:(dc + 1) * 512], ot)

                oa_sched = {2: [0], 3: [1], 4: [2, 3], 5: [4], 6: [5, 6],
                            7: [7], 8: [8, 9], 9: [10], 10: [11, 12],
                            11: [13]}
                psoH1_sched = {12: [0, 1, 2, 3], 13: [4, 5, 6, 7],
                               14: [8, 9, 10, 11], 15: [12, 13, 14, 15]}
                pv_extra = {14: [12], 15: [13]}
                for h in range(4, 16):
                    fillers = [lambda g=h - 4: emit_pv_qc(g, 0),
                               lambda g=h - 4: emit_pv_qc(g, 1)]
                    for idx in pv_extra.get(h, []):
                        fillers += [lambda g=idx: emit_pv_qc(g, 0),
                                    lambda g=idx: emit_pv_qc(g, 1)]
                    for idx in oa_sched.get(h - 2, []):
                        fillers.append(
                            lambda i=idx: emit_oa(i, xqb_sb, woA_sb))
                    for idx in psoH1_sched.get(h, []):
                        fillers.append(lambda i=idx: emit_pso(i, 0, True))
                    emit_head(h, fillers)
                # tail: remaining oa + PV, then ctx @ Wo_bot quads 2,3
                emit_oa(14, xqb_sb, woA_sb)
                emit_pv(14)
                emit_oa(15, xqb_sb, woA_sb)
                emit_pv(15)
                for idx in range(16):
                    emit_pso(idx, 2, False)

    nc.finalize()
    return nc


_NC_CACHE = {}


def _get_nc():
    if "nc" not in _NC_CACHE:
        _NC_CACHE["nc"] = build_nc()
    return _NC_CACHE["nc"]


# head h -> (quad, lh, par); dout column order for wv2/bvb: lh-major
def _v_col_perm():
    cols = []
    for lh in range(2):
        for q4 in range(4):
            for par in range(2):
                h = 4 * q4 + 2 * lh + par
                cols.extend(range(h * 64, h * 64 + 64))
    return np.array(cols)


def _prep_shared(wq_w, wq_b, wk_w, wk_b, wv_w, wv_b, wo_w, wo_b):
    def wtiles(w):
        # [p, t, i, n] = SW*w[(2t+i)*128+p, n]
        return np.ascontiguousarray(
            (SW * w).reshape(4, 2, 128, D).transpose(2, 0, 1, 3)).astype(NP8)

    def btile(b):
        # [p, a] = SW*b[a*128+p]
        return np.ascontiguousarray((SW * b).reshape(8, 128).T).astype(np.float32)

    perm = _v_col_perm()
    # wo2n[p, q, i, :] = SW*wo_bot[(4q + 2*(p//64) + i)*64 + p%64, :]
    wo_bot = (SW * wo_w[D:]).reshape(16, 64, D)
    wo2n = np.empty((128, 4, 2, D), dtype=np.float32)
    for q4 in range(4):
        for lh in range(2):
            for par in range(2):
                h = 4 * q4 + 2 * lh + par
                wo2n[lh * 64:lh * 64 + 64, q4, par, :] = wo_bot[h]

    return dict(
        wq2=wtiles(wq_w), wk2=wtiles(wk_w), wv2=wtiles(wv_w[:, perm]),
        bq2=btile(wq_b), bk2=btile(wk_b),
        bvb=np.ascontiguousarray(
            np.broadcast_to(SW * wv_b[perm], (128, D))).astype(np.float32),
        woA=np.ascontiguousarray(
            wo_w[:D].reshape(8, 128, D).transpose(1, 0, 2)).astype(NPB),
        wo2=np.ascontiguousarray(wo2n).astype(NP8),
        bob=np.ascontiguousarray(
            np.broadcast_to(wo_b, (128, D))).astype(np.float32),
    )


def _merge_in_map(core, shared, mb):
    return {
        "mq": np.ascontiguousarray(
            np.stack([shared["wq2"], core["xq8"]], axis=1)),
        "mk": np.ascontiguousarray(
            np.stack([shared["wk2"], core["xk8"]], axis=1)),
        "mv": np.ascontiguousarray(
            np.stack([shared["wv2"], core["xv8"]], axis=1)),
        "mb3": np.ascontiguousarray(
            np.concatenate([shared["bq2"], shared["bk2"], mb], axis=1)),
        "mba": np.ascontiguousarray(
            np.stack([core["xqb"], shared["woA"]], axis=1)),
        "bvb": shared["bvb"], "wo2": shared["wo2"], "bob": shared["bob"],
    }


def kernel(**inputs):
    global LAST_EXEC_NS
    v = np.asarray(inputs["v"], np.float32)
    k = np.asarray(inputs["k"], np.float32)
    q_in = np.asarray(inputs["q_in"], np.float32)
    mask = np.asarray(inputs["mask"], np.float32)

    shared = _prep_shared(
        np.asarray(inputs["wq_w"], np.float32), np.asarray(inputs["wq_b"], np.float32),
        np.asarray(inputs["wk_w"], np.float32), np.asarray(inputs["wk_b"], np.float32),
        np.asarray(inputs["wv_w"], np.float32), np.asarray(inputs["wv_b"], np.float32),
        np.asarray(inputs["wo_w"], np.float32), np.asarray(inputs["wo_b"], np.float32))

    def xtiles(x):
        return np.ascontiguousarray(
            x.T.reshape(4, 2, 128, S).transpose(2, 0, 1, 3)).astype(NP8)

    in_maps = []
    for b in range(B):
        mb = np.ascontiguousarray(
            (-1e9 * mask[b, 0, 0]).reshape(8, 128).T).astype(np.float32)
        xqb = np.ascontiguousarray(
            q_in[b].T.reshape(8, 128, S).transpose(1, 0, 2)).astype(NPB)
        core = {"xq8": xtiles(q_in[b]), "xk8": xtiles(k[b]),
                "xv8": xtiles(v[b]), "xqb": xqb}
        in_maps.append(_merge_in_map(core, shared, mb))

    nc = _get_nc()
    globals()["_LAST_NC"] = nc
    trace = os.environ.get("MHA_TRACE", "0") == "1"
    res = run_bass_kernel_spmd(nc, in_maps, core_ids=list(range(B)), trace=trace)
    LAST_EXEC_NS = res.exec_time_ns
    globals()["LAST_RES"] = res
    return np.stack([r["out"] for r in res.results], axis=0)
